# revision 2
# baseline (speedup 1.0000x reference)
"""Trainium2 Bass kernel for the segmented-attention block.

Reference computation (per batch row b of x [B, S*D]):
    xs = x[b].reshape(S, D)
    q_s = xs[s] @ Q[s]; k_s = xs[s] @ K[s]; v_s = xs[s] @ V[s]   (per segment)
    scores[s] = dot(q_s, k_s)
    w = scores / ||scores||_2
    y[b] = sum_s w[s] * v_s            -> [E]

Key algebraic fold: scores[s] = x_s^T (Q_s K_s^T) x_s, so precompute
G_s = Q_s @ K_s^T on the host (exact fp32 contraction over E) and on
device compute z = x_s @ G_s (matmul) followed by an elementwise
dot with x_s on the Vector engine. This removes one of the three
projection matmul passes: device tensor work drops from 3*B*D*E*S to
2*B*D*E*S FLOPs.

Sharding: data-parallel over B across 8 cores (512 rows each), G/V
replicated. Host pre-packs every DMA source so each partition reads one
contiguous 4KB line per segment; math is bf16 in, fp32 accumulation.

Self-contained: hardcodes all shapes; imports concourse from the system
install.
"""

import sys

import numpy as np
import ml_dtypes

for _p in ("/opt/trn_rl_repo",):
    if _p not in sys.path:
        sys.path.append(_p)

B, S, D, E = 4096, 32, 512, 512
NCORES = 8
BLOC = B // NCORES  # rows per core
P = 128             # partitions
DC = D // P         # contraction chunks per segment
BT = BLOC // P      # output row tiles per core

_BF16 = ml_dtypes.bfloat16

_nc_cache = None


def _build_bass():
    import concourse.bass as bass
    import concourse.mybir as mybir
    import concourse.tile as tile
    from concourse import bacc
    from concourse.bass import ts
    from contextlib import ExitStack

    fp32 = mybir.dt.float32
    bf16 = mybir.dt.bfloat16
    mult = mybir.AluOpType.mult
    add = mybir.AluOpType.add

    # Bacc (not raw Bass): its compile() pass splits multi-waits into
    # EventSemaphore insts (TRN2 allows 1 wait/inst) and lowers ISA ops.
    nc = bacc.Bacc("TRN2", debug=False)

    # All DRAM inputs are host-packed so a [P, ...] DMA slice reads one
    # contiguous run per partition.
    xt = nc.dram_tensor("xt", [S, P, DC, BLOC], bf16, kind="ExternalInput")  # x^T
    xr = nc.dram_tensor("xr", [S, P, BT, D], bf16, kind="ExternalInput")     # x rows
    gd = nc.dram_tensor("gd", [S, P, DC, D], bf16, kind="ExternalInput")     # Q K^T
    vd = nc.dram_tensor("vd", [S, P, DC, E], bf16, kind="ExternalInput")
    yd = nc.dram_tensor("y", [BLOC, E], fp32, kind="ExternalOutput")

    yr = yd.rearrange("(t p) e -> t p e", p=P)

    with ExitStack() as ctx:
        tc = ctx.enter_context(tile.TileContext(nc))
        singles = ctx.enter_context(tc.tile_pool(name="singles", bufs=1))
        wpool = ctx.enter_context(tc.tile_pool(name="wmat", bufs=4))
        xrpool = ctx.enter_context(tc.tile_pool(name="xrows", bufs=3))
        spool = ctx.enter_context(tc.tile_pool(name="scratch", bufs=3))
        psum = ctx.enter_context(tc.tile_pool(name="psum", bufs=2, space="PSUM"))
        psum3 = ctx.enter_context(tc.tile_pool(name="psum3", bufs=4, space="PSUM"))

        # Residents: x^T for all segments (bf16, 128KB/partition), scores,
        # weights, output accumulator.
        xts = singles.tile([P, S, DC, BLOC], bf16)
        scores = singles.tile([P, BT, S], fp32)
        wts = singles.tile([P, BT, S], fp32)
        y_sb = singles.tile([P, BT, E], fp32)

        nc.vector.memset(y_sb, 0.0)

        # ---- pass 1: z = x G, scores = sum(z * x) ---------------------
        for s in range(S):
            g_sb = wpool.tile([P, DC, D], bf16, tag="w")
            xr_sb = xrpool.tile([P, BT, D], bf16, tag="xr")
            if s == 0:
                # chunk the very first loads so the first matmul can start
                # after ~256KB instead of 1.5MB
                for c in range(DC):
                    nc.sync.dma_start(out=xts[:, s, c], in_=xt[s, :, c])
                    nc.sync.dma_start(out=g_sb[:, c], in_=gd[s, :, c])
                nc.sync.dma_start(out=xr_sb, in_=xr[s])
            else:
                nc.sync.dma_start(out=xts[:, s], in_=xt[s])
                nc.sync.dma_start(out=g_sb, in_=gd[s])
                nc.sync.dma_start(out=xr_sb, in_=xr[s])
            for bt in range(BT):
                z_ps = psum.tile([P, D], fp32, tag="zps", bufs=4)
                for c in range(DC):
                    nc.tensor.matmul(
                        z_ps,
                        xts[:, s, c, ts(bt, P)],
                        g_sb[:, c],
                        start=(c == 0),
                        stop=(c == DC - 1),
                    )
                # scores[:, bt, s] = sum_d z*x ; z is in PSUM, x-rows in
                # SBUF, so the DVE can read both directly.
                junk = spool.tile([P, D], fp32, tag="junk")
                nc.vector.scalar_tensor_tensor(
                    out=junk,
                    in0=z_ps,
                    scalar=1.0,
                    in1=xr_sb[:, bt],
                    op0=mult,
                    op1=mult,
                    accum_out=scores[:, bt, s : s + 1],
                )

        # ---- normalize scores -> weights ------------------------------
        for bt in range(BT):
            sq = spool.tile([P, S], fp32, tag="normtmp")
            nsq = spool.tile([P, 1], fp32, tag="nsq")
            nc.vector.scalar_tensor_tensor(
                out=sq,
                in0=scores[:, bt],
                scalar=1.0,
                in1=scores[:, bt],
                op0=mult,
                op1=mult,
                accum_out=nsq,
            )
            nrm = spool.tile([P, 1], fp32, tag="nrm")
            nc.scalar.sqrt(nrm, nsq)
            rcp = spool.tile([P, 1], fp32, tag="rcp")
            nc.vector.reciprocal(rcp, nrm)
            nc.vector.tensor_scalar_mul(wts[:, bt], scores[:, bt], rcp)

        # ---- pass 2: v projection + weighted accumulation -------------
        for s in range(S):
            v_sb = wpool.tile([P, DC, E], bf16, tag="w")
            nc.sync.dma_start(out=v_sb, in_=vd[s])
            for bt in range(BT):
                v_ps = psum3.tile([P, E], fp32, tag="vps", bufs=4)
                for c in range(DC):
                    nc.tensor.matmul(
                        v_ps,
                        xts[:, s, c, ts(bt, P)],
                        v_sb[:, c],
                        start=(c == 0),
                        stop=(c == DC - 1),
                    )
                # y[:, bt] += w[:, bt, s] * v
                nc.vector.scalar_tensor_tensor(
                    out=y_sb[:, bt],
                    in0=v_ps,
                    scalar=wts[:, bt, s : s + 1],
                    in1=y_sb[:, bt],
                    op0=mult,
                    op1=add,
                )

        # ---- store ----------------------------------------------------
        for bt in range(BT):
            nc.sync.dma_start(out=yr[bt], in_=y_sb[:, bt])

    # Run Bacc's compile passes (wait-splitting, ISA lowering, reg alloc).
    nc.finalize()
    return nc


def _get_nc():
    global _nc_cache
    if _nc_cache is None:
        _nc_cache = _build_bass()
    return _nc_cache


def _prep_in_maps(x, Q, K, V):
    x = np.asarray(x, dtype=np.float32)
    Q = np.ascontiguousarray(np.asarray(Q, dtype=np.float32))
    K = np.ascontiguousarray(np.asarray(K, dtype=np.float32))
    V = np.ascontiguousarray(np.asarray(V, dtype=np.float32))

    # Fold Q K^T -> G per segment (fp32, exact contraction over E).
    G = np.matmul(Q, K.transpose(0, 2, 1))  # [S, D, D]
    # Pack for contiguous per-partition DMA lines: [S, P, DC, D]
    gp = np.ascontiguousarray(
        G.reshape(S, DC, P, D).transpose(0, 2, 1, 3)
    ).astype(_BF16)
    vp = np.ascontiguousarray(
        V.reshape(S, DC, P, E).transpose(0, 2, 1, 3)
    ).astype(_BF16)

    in_maps = []
    for c in range(NCORES):
        xc = x[c * BLOC : (c + 1) * BLOC].reshape(BLOC, S, D).astype(_BF16)
        # x^T packed [S, P, DC, BLOC]: [s,p,c,b] = x[b, s, c*P+p]
        xtc = np.ascontiguousarray(
            xc.reshape(BLOC, S, DC, P).transpose(1, 3, 2, 0)
        )
        # x rows packed [S, P, BT, D]: [s,p,t,d] = x[t*P+p, s, d]
        xrc = np.ascontiguousarray(
            xc.reshape(BT, P, S, D).transpose(2, 1, 0, 3)
        )
        in_maps.append({"xt": xtc, "xr": xrc, "gd": gp, "vd": vp})
    return in_maps


def _run(in_maps, trace=False):
    from concourse.bass_utils import run_bass_kernel_spmd

    nc = _get_nc()
    res = run_bass_kernel_spmd(nc, in_maps, core_ids=list(range(NCORES)), trace=trace)
    y = np.concatenate([r["y"] for r in res.results], axis=0)
    return y, res


def kernel(x=None, Q=None, K=None, V=None, **_ignored):
    in_maps = _prep_in_maps(x, Q, K, V)
    y, _ = _run(in_maps, trace=False)
    return y


def kernel_traced(x, Q, K, V):
    in_maps = _prep_in_maps(x, Q, K, V)
    return _run(in_maps, trace=True)


# revision 3
# speedup vs baseline: 1.0854x; 1.0854x over previous
"""Trainium2 Bass kernel for the segmented-attention block.

Reference computation (per batch row b of x [B, S*D]):
    xs = x[b].reshape(S, D)
    q_s = xs[s] @ Q[s]; k_s = xs[s] @ K[s]; v_s = xs[s] @ V[s]   (per segment)
    scores[s] = dot(q_s, k_s)
    w = scores / ||scores||_2
    y[b] = sum_s w[s] * v_s            -> [E]

Two algebraic folds:
  1. scores[s] = x_s^T (Q_s K_s^T) x_s, so G_s = Q_s @ K_s^T is
     precomputed on the host (exact fp32 contraction over E); on device
     z = x_s @ G_s (matmul) then an elementwise dot with x_s on the DVE.
  2. Normalization is deferred: y_raw = sum_s scores[s] * v_s is
     accumulated with RAW scores segment by segment, and y = y_raw /
     ||scores|| once at the end. This removes the all-segments barrier,
     so z-matmuls, score dots, v-matmuls and y-accumulation all
     interleave in ONE pass per segment — DMA traffic is spread evenly
     and no x residency in SBUF is needed.

Sharding: data-parallel over B across 8 cores (512 rows each), G/V
replicated. Host pre-packs every DMA source so each partition reads one
contiguous 4KB line per segment; math is bf16 in, fp32 accumulation.

Self-contained: hardcodes all shapes; imports concourse from the system
install.
"""

import sys

import numpy as np
import ml_dtypes

for _p in ("/opt/trn_rl_repo",):
    if _p not in sys.path:
        sys.path.append(_p)

B, S, D, E = 4096, 32, 512, 512
NCORES = 8
BLOC = B // NCORES  # rows per core
P = 128             # partitions
DC = D // P         # contraction chunks per segment
BT = BLOC // P      # output row tiles per core

_BF16 = ml_dtypes.bfloat16

_nc_cache = None


def _build_bass():
    import concourse.bass as bass
    import concourse.mybir as mybir
    import concourse.tile as tile
    from concourse import bacc
    from concourse.bass import ts
    from contextlib import ExitStack

    fp32 = mybir.dt.float32
    bf16 = mybir.dt.bfloat16
    mult = mybir.AluOpType.mult
    add = mybir.AluOpType.add

    # Bacc (not raw Bass): its compile() pass splits multi-waits into
    # EventSemaphore insts (TRN2 allows 1 wait/inst) and lowers ISA ops.
    nc = bacc.Bacc("TRN2", debug=False)

    # All DRAM inputs are host-packed so a [P, ...] DMA slice reads one
    # contiguous run per partition.
    xt = nc.dram_tensor("xt", [S, P, DC, BLOC], bf16, kind="ExternalInput")  # x^T
    xr = nc.dram_tensor("xr", [S, P, BT, D], bf16, kind="ExternalInput")     # x rows
    gd = nc.dram_tensor("gd", [S, P, DC, D], bf16, kind="ExternalInput")     # Q K^T
    vd = nc.dram_tensor("vd", [S, P, DC, E], bf16, kind="ExternalInput")
    yd = nc.dram_tensor("y", [BLOC, E], fp32, kind="ExternalOutput")

    yr = yd.rearrange("(t p) e -> t p e", p=P)

    with ExitStack() as ctx:
        tc = ctx.enter_context(tile.TileContext(nc))
        singles = ctx.enter_context(tc.tile_pool(name="singles", bufs=1))
        xtpool = ctx.enter_context(tc.tile_pool(name="xtp", bufs=6))
        gpool = ctx.enter_context(tc.tile_pool(name="gp", bufs=6))
        vpool = ctx.enter_context(tc.tile_pool(name="vp", bufs=6))
        xrpool = ctx.enter_context(tc.tile_pool(name="xrp", bufs=6))
        spool = ctx.enter_context(tc.tile_pool(name="scratch", bufs=3))
        psum = ctx.enter_context(tc.tile_pool(name="psum", bufs=2, space="PSUM"))
        psum3 = ctx.enter_context(tc.tile_pool(name="psum3", bufs=2, space="PSUM"))

        # Residents: raw scores, y accumulator, rescaled output.
        scores = singles.tile([P, BT, S], fp32)
        y_sb = singles.tile([P, BT, E], fp32)
        y_out = singles.tile([P, BT, E], fp32)

        nc.vector.memset(y_sb, 0.0)

        # ---- single pass: z = x G, scores = sum(z*x), v = x V, --------
        # ---- y_raw += scores * v --------------------------------------
        for s in range(S):
            xt_sb = xtpool.tile([P, DC, BLOC], bf16, tag="xt")
            g_sb = gpool.tile([P, DC, D], bf16, tag="g")
            xr_sb = xrpool.tile([P, BT, D], bf16, tag="xr")
            v_sb = vpool.tile([P, DC, E], bf16, tag="v")
            if s == 0:
                # chunk the very first loads so the first matmul can start
                # after ~256KB instead of 2MB
                for c in range(DC):
                    nc.sync.dma_start(out=xt_sb[:, c], in_=xt[s, :, c])
                    nc.sync.dma_start(out=g_sb[:, c], in_=gd[s, :, c])
                nc.sync.dma_start(out=xr_sb, in_=xr[s])
                nc.sync.dma_start(out=v_sb, in_=vd[s])
            else:
                nc.sync.dma_start(out=xt_sb, in_=xt[s])
                nc.sync.dma_start(out=g_sb, in_=gd[s])
                nc.sync.dma_start(out=xr_sb, in_=xr[s])
                nc.sync.dma_start(out=v_sb, in_=vd[s])
            for bt in range(BT):
                z_ps = psum.tile([P, D], fp32, tag="zps", bufs=4)
                for c in range(DC):
                    nc.tensor.matmul(
                        z_ps,
                        xt_sb[:, c, ts(bt, P)],
                        g_sb[:, c],
                        start=(c == 0),
                        stop=(c == DC - 1),
                    )
                # scores[:, bt, s] = sum_d z*x ; z in PSUM, x-rows in SBUF
                junk = spool.tile([P, D], bf16, tag="junk")
                nc.vector.scalar_tensor_tensor(
                    out=junk,
                    in0=z_ps,
                    scalar=1.0,
                    in1=xr_sb[:, bt],
                    op0=mult,
                    op1=mult,
                    accum_out=scores[:, bt, s : s + 1],
                )
                v_ps = psum3.tile([P, E], fp32, tag="vps", bufs=4)
                for c in range(DC):
                    nc.tensor.matmul(
                        v_ps,
                        xt_sb[:, c, ts(bt, P)],
                        v_sb[:, c],
                        start=(c == 0),
                        stop=(c == DC - 1),
                    )
                # y_raw[:, bt] += scores[:, bt, s] * v   (raw, unnormalized)
                nc.vector.scalar_tensor_tensor(
                    out=y_sb[:, bt],
                    in0=v_ps,
                    scalar=scores[:, bt, s : s + 1],
                    in1=y_sb[:, bt],
                    op0=mult,
                    op1=add,
                )

        # ---- normalize: y = y_raw / ||scores|| ; store ----------------
        for bt in range(BT):
            sq = spool.tile([P, S], fp32, tag="normtmp")
            nsq = spool.tile([P, 1], fp32, tag="nsq")
            nc.vector.scalar_tensor_tensor(
                out=sq,
                in0=scores[:, bt],
                scalar=1.0,
                in1=scores[:, bt],
                op0=mult,
                op1=mult,
                accum_out=nsq,
            )
            nrm = spool.tile([P, 1], fp32, tag="nrm")
            nc.scalar.sqrt(nrm, nsq)
            rcp = spool.tile([P, 1], fp32, tag="rcp")
            nc.vector.reciprocal(rcp, nrm)
            nc.vector.tensor_scalar_mul(y_out[:, bt], y_sb[:, bt], rcp)
            nc.sync.dma_start(out=yr[bt], in_=y_out[:, bt])

    # Run Bacc's compile passes (wait-splitting, ISA lowering, reg alloc).
    nc.finalize()
    return nc


def _get_nc():
    global _nc_cache
    if _nc_cache is None:
        _nc_cache = _build_bass()
    return _nc_cache


def _prep_in_maps(x, Q, K, V):
    x = np.asarray(x, dtype=np.float32)
    Q = np.ascontiguousarray(np.asarray(Q, dtype=np.float32))
    K = np.ascontiguousarray(np.asarray(K, dtype=np.float32))
    V = np.ascontiguousarray(np.asarray(V, dtype=np.float32))

    # Fold Q K^T -> G per segment (fp32, exact contraction over E).
    G = np.matmul(Q, K.transpose(0, 2, 1))  # [S, D, D]
    # Pack for contiguous per-partition DMA lines: [S, P, DC, D]
    gp = np.ascontiguousarray(
        G.reshape(S, DC, P, D).transpose(0, 2, 1, 3)
    ).astype(_BF16)
    vp = np.ascontiguousarray(
        V.reshape(S, DC, P, E).transpose(0, 2, 1, 3)
    ).astype(_BF16)

    in_maps = []
    for c in range(NCORES):
        xc = x[c * BLOC : (c + 1) * BLOC].reshape(BLOC, S, D).astype(_BF16)
        # x^T packed [S, P, DC, BLOC]: [s,p,c,b] = x[b, s, c*P+p]
        xtc = np.ascontiguousarray(
            xc.reshape(BLOC, S, DC, P).transpose(1, 3, 2, 0)
        )
        # x rows packed [S, P, BT, D]: [s,p,t,d] = x[t*P+p, s, d]
        xrc = np.ascontiguousarray(
            xc.reshape(BT, P, S, D).transpose(2, 1, 0, 3)
        )
        in_maps.append({"xt": xtc, "xr": xrc, "gd": gp, "vd": vp})
    return in_maps


def _run(in_maps, trace=False):
    from concourse.bass_utils import run_bass_kernel_spmd

    nc = _get_nc()
    res = run_bass_kernel_spmd(nc, in_maps, core_ids=list(range(NCORES)), trace=trace)
    y = np.concatenate([r["y"] for r in res.results], axis=0)
    return y, res


def kernel(x=None, Q=None, K=None, V=None, **_ignored):
    in_maps = _prep_in_maps(x, Q, K, V)
    y, _ = _run(in_maps, trace=False)
    return y


def kernel_traced(x, Q, K, V):
    in_maps = _prep_in_maps(x, Q, K, V)
    return _run(in_maps, trace=True)


# revision 8
# speedup vs baseline: 1.1525x; 1.0618x over previous
"""Trainium2 Bass kernel for the segmented-attention block.

Reference computation (per batch row b of x [B, S*D]):
    xs = x[b].reshape(S, D)
    q_s = xs[s] @ Q[s]; k_s = xs[s] @ K[s]; v_s = xs[s] @ V[s]   (per segment)
    scores[s] = dot(q_s, k_s)
    w = scores / ||scores||_2
    y[b] = sum_s w[s] * v_s            -> [E]

Three algebraic folds:
  1. scores[s] = x_s^T (Q_s K_s^T) x_s, so G_s = Q_s @ K_s^T is
     precomputed on the host (exact fp32 contraction over E); on device
     z = x_s @ G_s (matmul) then an elementwise dot with x_s on the DVE.
  2. Only the symmetric part A = (G+G^T)/2 matters for x^T G x, so the
     host folds A into a block-upper-triangular U at 256-column
     granularity (off-diagonal block doubled): the z matmul needs only
     6 of 8 [128,256] blocks -> 25% less z tensor work and G traffic.
  3. Normalization is deferred: y_raw = sum_s scores[s] * v_s is
     accumulated with RAW scores segment by segment, and y = y_raw /
     ||scores|| once at the end. This removes the all-segments barrier,
     so z-matmuls, score dots, v-matmuls and y-accumulation all
     interleave in ONE pass per segment — DMA traffic is spread evenly
     and no x residency in SBUF is needed.

Engine balance: the score dot is staged PSUM->SBUF(bf16) by the idle
Scalar engine so the DVE runs it in 16-bit; ~25 dummy matmuls at kernel
start warm the PE HAM clock gate during the first DMA wait.

Sharding: data-parallel over B across 8 cores (512 rows each), G/V
replicated. Host pre-packs every DMA source so each partition reads one
contiguous 4KB line per segment; math is bf16 in, fp32 accumulation.

Self-contained: hardcodes all shapes; imports concourse from the system
install.
"""

import sys

import numpy as np
import ml_dtypes

for _p in ("/opt/trn_rl_repo",):
    if _p not in sys.path:
        sys.path.append(_p)

B, S, D, E = 4096, 32, 512, 512
NCORES = 8
BLOC = B // NCORES  # rows per core
P = 128             # partitions
DC = D // P         # contraction chunks per segment
BT = BLOC // P      # output row tiles per core

_BF16 = ml_dtypes.bfloat16

_nc_cache = None


def _build_bass():
    import concourse.bass as bass
    import concourse.mybir as mybir
    import concourse.tile as tile
    from concourse import bacc
    from concourse.bass import ts
    from contextlib import ExitStack

    fp32 = mybir.dt.float32
    bf16 = mybir.dt.bfloat16
    mult = mybir.AluOpType.mult
    add = mybir.AluOpType.add

    # Bacc (not raw Bass): its compile() pass splits multi-waits into
    # EventSemaphore insts (TRN2 allows 1 wait/inst) and lowers ISA ops.
    nc = bacc.Bacc("TRN2", debug=False)

    # All DRAM inputs are host-packed so a [P, ...] DMA slice reads one
    # contiguous run per partition.
    xt = nc.dram_tensor("xt", [S, P, DC, BLOC], bf16, kind="ExternalInput")  # x^T
    xr = nc.dram_tensor("xr", [S, P, BT, D], bf16, kind="ExternalInput")     # x rows
    ud = nc.dram_tensor("ud", [S, P, 6, 256], bf16, kind="ExternalInput")    # tri(QK^T)
    vd = nc.dram_tensor("vd", [S, P, DC, E], bf16, kind="ExternalInput")
    yd = nc.dram_tensor("y", [BLOC, E], fp32, kind="ExternalOutput")

    yr = yd.rearrange("(t p) e -> t p e", p=P)

    with ExitStack() as ctx:
        tc = ctx.enter_context(tile.TileContext(nc))
        singles = ctx.enter_context(tc.tile_pool(name="singles", bufs=1))
        xtpool = ctx.enter_context(tc.tile_pool(name="xtp", bufs=6))
        gpool = ctx.enter_context(tc.tile_pool(name="gp", bufs=6))
        vpool = ctx.enter_context(tc.tile_pool(name="vp", bufs=6))
        xrpool = ctx.enter_context(tc.tile_pool(name="xrp", bufs=6))
        spool = ctx.enter_context(tc.tile_pool(name="scratch", bufs=3))
        psum = ctx.enter_context(tc.tile_pool(name="psum", bufs=2, space="PSUM"))
        psum3 = ctx.enter_context(tc.tile_pool(name="psum3", bufs=2, space="PSUM"))

        # Residents: raw scores, y accumulator, rescaled output.
        scores = singles.tile([P, BT, S], fp32)
        y_sb = singles.tile([P, BT, E], fp32)
        y_out = singles.tile([P, BT, E], fp32)
        warm_sb = singles.tile([P, P], bf16)

        # Warm the PE HAM clock gate (~3.4us of activity flips 1.2GHz ->
        # 2.4GHz) while the first DMAs are in flight.
        nc.vector.memset(warm_sb, 0.0)
        nc.vector.memset(y_sb, 0.0)
        warm_ps = psum.tile([P, P], fp32, tag="warm", bufs=1)
        for _ in range(25):
            nc.tensor.matmul(warm_ps, warm_sb, warm_sb, start=True, stop=True)

        # ---- single pass: z = x U, scores = sum(z*x), v = x V, --------
        # ---- y_raw += scores * v --------------------------------------
        for s in range(S):
            xt_sb = xtpool.tile([P, DC, BLOC], bf16, tag="xt")
            u_sb = gpool.tile([P, 6, 256], bf16, tag="g")
            xr_sb = xrpool.tile([P, BT, D], bf16, tag="xr")
            v_sb = vpool.tile([P, DC, E], bf16, tag="v")
            if s == 0:
                # chunk the very first loads so the first matmul can start
                # as early as possible
                nc.sync.dma_start(out=xt_sb[:, 0], in_=xt[s, :, 0])
                nc.sync.dma_start(out=xt_sb[:, 1], in_=xt[s, :, 1])
                nc.sync.dma_start(out=u_sb[:, 0:2], in_=ud[s, :, 0:2])
                nc.sync.dma_start(out=xt_sb[:, 2], in_=xt[s, :, 2])
                nc.sync.dma_start(out=xt_sb[:, 3], in_=xt[s, :, 3])
                nc.sync.dma_start(out=u_sb[:, 2:6], in_=ud[s, :, 2:6])
                nc.sync.dma_start(out=xr_sb, in_=xr[s])
                nc.sync.dma_start(out=v_sb, in_=vd[s])
            else:
                nc.sync.dma_start(out=xt_sb, in_=xt[s])
                nc.sync.dma_start(out=u_sb, in_=ud[s])
                nc.sync.dma_start(out=xr_sb, in_=xr[s])
                nc.sync.dma_start(out=v_sb, in_=vd[s])
            for bt in range(BT):
                z_ps = psum.tile([P, D], fp32, tag="zps", bufs=3)
                # upper-left block column: cols 0:256, d-chunks 0,1
                for c in range(2):
                    nc.tensor.matmul(
                        z_ps[:, 0:256],
                        xt_sb[:, c, ts(bt, P)],
                        u_sb[:, c],
                        start=(c == 0),
                        stop=(c == 1),
                    )
                # right block column: cols 256:512, d-chunks 0..3
                for c in range(DC):
                    nc.tensor.matmul(
                        z_ps[:, 256:512],
                        xt_sb[:, c, ts(bt, P)],
                        u_sb[:, 2 + c],
                        start=(c == 0),
                        stop=(c == DC - 1),
                    )
                # stage z to SBUF bf16 on the idle Scalar engine so the
                # DVE score dot runs 16-bit
                z_cp = spool.tile([P, D], bf16, tag="zcp")
                nc.scalar.copy(z_cp, z_ps)
                # scores[:, bt, s] = sum_d z*x
                junk = spool.tile([P, D], bf16, tag="junk")
                nc.vector.scalar_tensor_tensor(
                    out=junk,
                    in0=z_cp,
                    scalar=1.0,
                    in1=xr_sb[:, bt],
                    op0=mult,
                    op1=mult,
                    accum_out=scores[:, bt, s : s + 1],
                )
                v_ps = psum3.tile([P, E], fp32, tag="vps", bufs=4)
                for c in range(DC):
                    nc.tensor.matmul(
                        v_ps,
                        xt_sb[:, c, ts(bt, P)],
                        v_sb[:, c],
                        start=(c == 0),
                        stop=(c == DC - 1),
                    )
                # y_raw[:, bt] += scores[:, bt, s] * v   (raw, unnormalized)
                nc.vector.scalar_tensor_tensor(
                    out=y_sb[:, bt],
                    in0=v_ps,
                    scalar=scores[:, bt, s : s + 1],
                    in1=y_sb[:, bt],
                    op0=mult,
                    op1=add,
                )

        # ---- normalize: y = y_raw / ||scores|| ; store ----------------
        for bt in range(BT):
            sq = spool.tile([P, S], fp32, tag="normtmp")
            nsq = spool.tile([P, 1], fp32, tag="nsq")
            nc.vector.scalar_tensor_tensor(
                out=sq,
                in0=scores[:, bt],
                scalar=1.0,
                in1=scores[:, bt],
                op0=mult,
                op1=mult,
                accum_out=nsq,
            )
            nrm = spool.tile([P, 1], fp32, tag="nrm")
            nc.scalar.sqrt(nrm, nsq)
            rcp = spool.tile([P, 1], fp32, tag="rcp")
            nc.vector.reciprocal(rcp, nrm)
            nc.vector.tensor_scalar_mul(y_out[:, bt], y_sb[:, bt], rcp)
            nc.sync.dma_start(out=yr[bt], in_=y_out[:, bt])

    # Run Bacc's compile passes (wait-splitting, ISA lowering, reg alloc).
    nc.finalize()
    return nc


def _get_nc():
    global _nc_cache
    if _nc_cache is None:
        _nc_cache = _build_bass()
    return _nc_cache


def _prep_in_maps(x, Q, K, V):
    x = np.asarray(x, dtype=np.float32)
    Q = np.ascontiguousarray(np.asarray(Q, dtype=np.float32))
    K = np.ascontiguousarray(np.asarray(K, dtype=np.float32))
    V = np.ascontiguousarray(np.asarray(V, dtype=np.float32))

    # Fold Q K^T -> G per segment (fp32, exact contraction over E), then
    # symmetrize and triangularize at 256-block granularity: x^T G x =
    # x^T A x with A = (G+G^T)/2 = x0'A00x0 + x1'A11x1 + x0'(2A01)x1.
    G = np.matmul(Q, K.transpose(0, 2, 1))  # [S, D, D]
    A = 0.5 * (G + G.transpose(0, 2, 1))
    # U blocks, [S, 6, P, 256]: slots 0,1 = A00 d-chunks (cols 0:256);
    # slots 2..5 = [2*A01; A11] d-chunks (cols 256:512)
    left = A[:, 0:256, 0:256].reshape(S, 2, P, 256)
    right = np.concatenate(
        [2.0 * A[:, 0:256, 256:512], A[:, 256:512, 256:512]], axis=1
    ).reshape(S, DC, P, 256)
    up = np.ascontiguousarray(
        np.concatenate([left, right], axis=1).transpose(0, 2, 1, 3)
    ).astype(_BF16)
    vp = np.ascontiguousarray(
        V.reshape(S, DC, P, E).transpose(0, 2, 1, 3)
    ).astype(_BF16)

    in_maps = []
    for c in range(NCORES):
        xc = x[c * BLOC : (c + 1) * BLOC].reshape(BLOC, S, D).astype(_BF16)
        # x^T packed [S, P, DC, BLOC]: [s,p,c,b] = x[b, s, c*P+p]
        xtc = np.ascontiguousarray(
            xc.reshape(BLOC, S, DC, P).transpose(1, 3, 2, 0)
        )
        # x rows packed [S, P, BT, D]: [s,p,t,d] = x[t*P+p, s, d]
        xrc = np.ascontiguousarray(
            xc.reshape(BT, P, S, D).transpose(2, 1, 0, 3)
        )
        in_maps.append({"xt": xtc, "xr": xrc, "ud": up, "vd": vp})
    return in_maps


def _run(in_maps, trace=False):
    from concourse.bass_utils import run_bass_kernel_spmd

    nc = _get_nc()
    res = run_bass_kernel_spmd(nc, in_maps, core_ids=list(range(NCORES)), trace=trace)
    y = np.concatenate([r["y"] for r in res.results], axis=0)
    return y, res


def kernel(x=None, Q=None, K=None, V=None, **_ignored):
    in_maps = _prep_in_maps(x, Q, K, V)
    y, _ = _run(in_maps, trace=False)
    return y


def kernel_traced(x, Q, K, V):
    in_maps = _prep_in_maps(x, Q, K, V)
    return _run(in_maps, trace=True)


# revision 13
# speedup vs baseline: 1.2060x; 1.0464x over previous
"""Trainium2 Bass kernel for the segmented-attention block.

Reference computation (per batch row b of x [B, S*D]):
    xs = x[b].reshape(S, D)
    q_s = xs[s] @ Q[s]; k_s = xs[s] @ K[s]; v_s = xs[s] @ V[s]   (per segment)
    scores[s] = dot(q_s, k_s)
    w = scores / ||scores||_2
    y[b] = sum_s w[s] * v_s            -> [E]

Three algebraic folds:
  1. scores[s] = x_s^T (Q_s K_s^T) x_s, so G_s = Q_s @ K_s^T is
     precomputed on the host (exact fp32 contraction over E); on device
     z = x_s @ G_s (matmul) then an elementwise dot with x_s on the DVE.
  2. Only the symmetric part A = (G+G^T)/2 matters for x^T G x, so the
     host folds A into a block-upper-triangular U at 256-column
     granularity (off-diagonal block doubled): the z matmul needs only
     6 of 8 [128,256] blocks -> 25% less z tensor work and G traffic.
  3. Normalization is deferred: y_raw = sum_s scores[s] * v_s is
     accumulated with RAW scores segment by segment, and y = y_raw /
     ||scores|| once at the end. This removes the all-segments barrier,
     so z-matmuls, score dots, v-matmuls and y-accumulation all
     interleave in ONE pass per segment — DMA traffic is spread evenly
     and no x residency in SBUF is needed.

Engine balance: ~40 dummy matmuls at kernel start warm the PE HAM
clock gate during the first DMA wait; the normalization/rescale tail is
pipelined into the last segment's loop with the rescale on the (idle)
Scalar engine, so the kernel ends ~one DMA store after the last matmul.

Sharding: data-parallel over B across 8 cores (512 rows each), G/V
replicated. Host pre-packs every DMA source so each partition reads one
contiguous 4KB line per segment; math is bf16 in, fp32 accumulation.

Self-contained: hardcodes all shapes; imports concourse from the system
install.
"""

import sys

import numpy as np
import ml_dtypes

for _p in ("/opt/trn_rl_repo",):
    if _p not in sys.path:
        sys.path.append(_p)

B, S, D, E = 4096, 32, 512, 512
NCORES = 8
BLOC = B // NCORES  # rows per core
P = 128             # partitions
DC = D // P         # contraction chunks per segment
BT = BLOC // P      # output row tiles per core

_BF16 = ml_dtypes.bfloat16

_nc_cache = None


def _build_bass():
    import concourse.bass as bass
    import concourse.mybir as mybir
    import concourse.tile as tile
    from concourse import bacc
    from concourse.bass import ts
    from contextlib import ExitStack

    fp32 = mybir.dt.float32
    bf16 = mybir.dt.bfloat16
    mult = mybir.AluOpType.mult
    add = mybir.AluOpType.add

    # Bacc (not raw Bass): its compile() pass splits multi-waits into
    # EventSemaphore insts (TRN2 allows 1 wait/inst) and lowers ISA ops.
    nc = bacc.Bacc("TRN2", debug=False)

    # All DRAM inputs are host-packed so a [P, ...] DMA slice reads one
    # contiguous run per partition.
    xt = nc.dram_tensor("xt", [S, P, DC, BLOC], bf16, kind="ExternalInput")  # x^T
    xr = nc.dram_tensor("xr", [S, P, BT, D], bf16, kind="ExternalInput")     # x rows
    ud = nc.dram_tensor("ud", [S, P, 6, 256], bf16, kind="ExternalInput")    # tri(QK^T)
    vd = nc.dram_tensor("vd", [S, P, DC, E], bf16, kind="ExternalInput")
    yd = nc.dram_tensor("y", [BLOC, E], fp32, kind="ExternalOutput")

    yr = yd.rearrange("(t p) e -> t p e", p=P)

    with ExitStack() as ctx:
        tc = ctx.enter_context(tile.TileContext(nc))
        singles = ctx.enter_context(tc.tile_pool(name="singles", bufs=1))
        xtpool = ctx.enter_context(tc.tile_pool(name="xtp", bufs=6))
        gpool = ctx.enter_context(tc.tile_pool(name="gp", bufs=6))
        vpool = ctx.enter_context(tc.tile_pool(name="vp", bufs=6))
        xrpool = ctx.enter_context(tc.tile_pool(name="xrp", bufs=6))
        spool = ctx.enter_context(tc.tile_pool(name="scratch", bufs=3))
        psum = ctx.enter_context(tc.tile_pool(name="psum", bufs=2, space="PSUM"))
        psum3 = ctx.enter_context(tc.tile_pool(name="psum3", bufs=2, space="PSUM"))

        # Residents: raw scores, y accumulator, rescaled output.
        scores = singles.tile([P, BT, S], fp32)
        y_sb = singles.tile([P, BT, E], fp32)
        y_out = singles.tile([P, BT, E], fp32)
        warm_sb = singles.tile([P, P], bf16)

        # Warm the PE HAM clock gate (~3.4us of activity flips 1.2GHz ->
        # 2.4GHz) while the first DMAs are in flight.
        nc.vector.memset(warm_sb, 0.0)
        nc.vector.memset(y_sb, 0.0)
        warm_ps = psum.tile([P, P], fp32, tag="warm", bufs=1)
        for _ in range(40):
            nc.tensor.matmul(warm_ps, warm_sb, warm_sb, start=True, stop=True)

        # ---- single pass: z = x U, scores = sum(z*x), v = x V, --------
        # ---- y_raw += scores * v --------------------------------------
        rcps = []
        for s in range(S):
            xt_sb = xtpool.tile([P, DC, BLOC], bf16, tag="xt")
            u_sb = gpool.tile([P, 6, 256], bf16, tag="g")
            xr_sb = xrpool.tile([P, BT, D], bf16, tag="xr")
            v_sb = vpool.tile([P, DC, E], bf16, tag="v")
            if s == 0:
                # chunk the very first loads so the first matmul can start
                # as early as possible
                nc.sync.dma_start(out=xt_sb[:, 0], in_=xt[s, :, 0])
                nc.sync.dma_start(out=xt_sb[:, 1], in_=xt[s, :, 1])
                nc.sync.dma_start(out=u_sb[:, 0:2], in_=ud[s, :, 0:2])
                nc.sync.dma_start(out=xt_sb[:, 2], in_=xt[s, :, 2])
                nc.sync.dma_start(out=xt_sb[:, 3], in_=xt[s, :, 3])
                nc.sync.dma_start(out=u_sb[:, 2:6], in_=ud[s, :, 2:6])
                nc.sync.dma_start(out=xr_sb, in_=xr[s])
                nc.sync.dma_start(out=v_sb, in_=vd[s])
            else:
                nc.sync.dma_start(out=xt_sb, in_=xt[s])
                nc.sync.dma_start(out=u_sb, in_=ud[s])
                nc.sync.dma_start(out=xr_sb, in_=xr[s])
                nc.sync.dma_start(out=v_sb, in_=vd[s])
            for bt in range(BT):
                z_ps = psum.tile([P, D], fp32, tag="zps", bufs=3)
                # upper-left block column: cols 0:256, d-chunks 0,1
                for c in range(2):
                    nc.tensor.matmul(
                        z_ps[:, 0:256],
                        xt_sb[:, c, ts(bt, P)],
                        u_sb[:, c],
                        start=(c == 0),
                        stop=(c == 1),
                    )
                # right block column: cols 256:512, d-chunks 0..3
                for c in range(DC):
                    nc.tensor.matmul(
                        z_ps[:, 256:512],
                        xt_sb[:, c, ts(bt, P)],
                        u_sb[:, 2 + c],
                        start=(c == 0),
                        stop=(c == DC - 1),
                    )
                # scores[:, bt, s] = sum_d z*x ; z in PSUM, x-rows in SBUF
                junk = spool.tile([P, D], bf16, tag="junk")
                nc.vector.scalar_tensor_tensor(
                    out=junk,
                    in0=z_ps,
                    scalar=1.0,
                    in1=xr_sb[:, bt],
                    op0=mult,
                    op1=mult,
                    accum_out=scores[:, bt, s : s + 1],
                )
                if s == S - 1:
                    # all scores for this bt are now in: fold the norm
                    # computation into the pipeline (DVE+Scalar) while
                    # the v matmuls below still run on the PE
                    sq = spool.tile([P, S], fp32, tag="normtmp")
                    nsq = spool.tile([P, 1], fp32, tag="nsq")
                    nc.vector.scalar_tensor_tensor(
                        out=sq,
                        in0=scores[:, bt],
                        scalar=1.0,
                        in1=scores[:, bt],
                        op0=mult,
                        op1=mult,
                        accum_out=nsq,
                    )
                    nrm = spool.tile([P, 1], fp32, tag="nrm")
                    nc.scalar.sqrt(nrm, nsq)
                    rcp = spool.tile([P, 1], fp32, tag="rcp")
                    nc.vector.reciprocal(rcp, nrm)
                    rcps.append(rcp)
                v_ps = psum3.tile([P, E], fp32, tag="vps", bufs=4)
                for c in range(DC):
                    nc.tensor.matmul(
                        v_ps,
                        xt_sb[:, c, ts(bt, P)],
                        v_sb[:, c],
                        start=(c == 0),
                        stop=(c == DC - 1),
                    )
                # y_raw[:, bt] += scores[:, bt, s] * v   (raw, unnormalized)
                nc.vector.scalar_tensor_tensor(
                    out=y_sb[:, bt],
                    in0=v_ps,
                    scalar=scores[:, bt, s : s + 1],
                    in1=y_sb[:, bt],
                    op0=mult,
                    op1=add,
                )
                if s == S - 1:
                    # y = y_raw / ||scores|| on the Scalar engine; store
                    nc.scalar.mul(y_out[:, bt], y_sb[:, bt], rcps[bt])
                    nc.sync.dma_start(out=yr[bt], in_=y_out[:, bt])

    # Run Bacc's compile passes (wait-splitting, ISA lowering, reg alloc).
    nc.finalize()
    return nc


def _get_nc():
    global _nc_cache
    if _nc_cache is None:
        _nc_cache = _build_bass()
    return _nc_cache


def _prep_in_maps(x, Q, K, V):
    x = np.asarray(x, dtype=np.float32)
    Q = np.ascontiguousarray(np.asarray(Q, dtype=np.float32))
    K = np.ascontiguousarray(np.asarray(K, dtype=np.float32))
    V = np.ascontiguousarray(np.asarray(V, dtype=np.float32))

    # Fold Q K^T -> G per segment (fp32, exact contraction over E), then
    # symmetrize and triangularize at 256-block granularity: x^T G x =
    # x^T A x with A = (G+G^T)/2 = x0'A00x0 + x1'A11x1 + x0'(2A01)x1.
    G = np.matmul(Q, K.transpose(0, 2, 1))  # [S, D, D]
    A = 0.5 * (G + G.transpose(0, 2, 1))
    # U blocks, [S, 6, P, 256]: slots 0,1 = A00 d-chunks (cols 0:256);
    # slots 2..5 = [2*A01; A11] d-chunks (cols 256:512)
    left = A[:, 0:256, 0:256].reshape(S, 2, P, 256)
    right = np.concatenate(
        [2.0 * A[:, 0:256, 256:512], A[:, 256:512, 256:512]], axis=1
    ).reshape(S, DC, P, 256)
    up = np.ascontiguousarray(
        np.concatenate([left, right], axis=1).transpose(0, 2, 1, 3)
    ).astype(_BF16)
    vp = np.ascontiguousarray(
        V.reshape(S, DC, P, E).transpose(0, 2, 1, 3)
    ).astype(_BF16)

    in_maps = []
    for c in range(NCORES):
        xc = x[c * BLOC : (c + 1) * BLOC].reshape(BLOC, S, D).astype(_BF16)
        # x^T packed [S, P, DC, BLOC]: [s,p,c,b] = x[b, s, c*P+p]
        xtc = np.ascontiguousarray(
            xc.reshape(BLOC, S, DC, P).transpose(1, 3, 2, 0)
        )
        # x rows packed [S, P, BT, D]: [s,p,t,d] = x[t*P+p, s, d]
        xrc = np.ascontiguousarray(
            xc.reshape(BT, P, S, D).transpose(2, 1, 0, 3)
        )
        in_maps.append({"xt": xtc, "xr": xrc, "ud": up, "vd": vp})
    return in_maps


def _run(in_maps, trace=False):
    from concourse.bass_utils import run_bass_kernel_spmd

    nc = _get_nc()
    res = run_bass_kernel_spmd(nc, in_maps, core_ids=list(range(NCORES)), trace=trace)
    y = np.concatenate([r["y"] for r in res.results], axis=0)
    return y, res


def kernel(x=None, Q=None, K=None, V=None, **_ignored):
    in_maps = _prep_in_maps(x, Q, K, V)
    y, _ = _run(in_maps, trace=False)
    return y


def kernel_traced(x, Q, K, V):
    in_maps = _prep_in_maps(x, Q, K, V)
    return _run(in_maps, trace=True)
